# revision 1
# baseline (speedup 1.0000x reference)
"""AGNNProp on 8 Trainium2 NeuronCores.

out[i] = sum_{e: row_e = i} softmax_i(beta * cos(x_i, x_col_e)) * x[col_e]
with self-loops added (segment softmax grouped by destination row).

Strategy (graph/data parallel per sharding hint):
 - Host: group edges by destination, deal destinations round-robin (snake
   ordering by degree) to 8 cores, pad each 128-destination tile to its max
   per-range degree.  Two overlapping int16 gather ranges ([0,32768) and
   [7232,40000)) balance per-destination counts.  Self-loops are NOT
   gathered: cos(self)=1 exactly, handled as an extra softmax column.
 - Device: node table [N, 256] bf16 rows = [x_bf16 | invnorm_f32 | pad]
   (512B, dma_gather-aligned); per destination tile: dma_gather neighbor
   rows, tile-level dot products on DVE, per-partition softmax, weighted
   aggregation on the TensorEngine (G_j as stationary, diag(w_j) moving),
   accumulating the transposed output in PSUM.
"""

import sys

sys.path.insert(0, "/opt/trn_rl_repo")

import numpy as np

N_NODES = 40000
N_EDGES = 640000
D = 128
NC = 8
P = 128
LO = 32768  # int16 index limit -> gather A covers [0, LO)
DPC = 5120  # padded destinations per core
TPC = DPC // P  # 40 tiles per core
NEG = -1.0e30
ACH = 16  # table-build staging chunk: rows per partition

BASE_B = N_NODES - LO  # 7232: gather B covers [7232, 40000), width 32768


# ---------------------------------------------------------------- host side


def _preprocess(edge_index):
    """Integer bookkeeping (no self loops here - device adds the self
    column analytically): per-destination neighbor lists, destination
    dealing, per-tile capacities with overlapping A/B gather ranges."""
    row = np.asarray(edge_index[0], dtype=np.int64)
    col = np.asarray(edge_index[1], dtype=np.int64)

    # sort edges by (dest, col): must-A cols first, flexible, must-B last
    perm = np.lexsort((col, row))
    row, col = row[perm], col[perm]

    deg = np.bincount(row, minlength=N_NODES)
    a_cnt = np.bincount(row[col < BASE_B], minlength=N_NODES)  # must-A
    b_cnt = np.bincount(row[col >= LO], minlength=N_NODES)  # must-B
    starts = np.zeros(N_NODES + 1, dtype=np.int64)
    np.cumsum(deg, out=starts[1:])

    # snake ordering: degree desc, a_cnt alternating asc/desc per group
    ud = np.sort(np.unique(deg))[::-1]
    parts = []
    for gi, dv in enumerate(ud):
        idx = np.where(deg == dv)[0]
        o2 = np.argsort(a_cnt[idx] if gi % 2 == 0 else -a_cnt[idx], kind="stable")
        parts.append(idx[o2])
    order = np.concatenate(parts)

    # deal: rank r -> core r%8, slot r//8;  pad slots get dest=-1
    dest = np.full((NC, DPC), -1, dtype=np.int64)
    for c in range(NC):
        got = order[c::NC]
        dest[c, : len(got)] = got

    # per-tile capacities + per-dest A counts (one SPMD graph: max over cores)
    KL = np.zeros(TPC, dtype=np.int64)
    KH = np.zeros(TPC, dtype=np.int64)
    nA = np.zeros(N_NODES, dtype=np.int64)  # cols assigned to gather A
    for t in range(TPC):
        d = dest[:, t * P : (t + 1) * P].ravel()
        d = d[d >= 0]
        if not len(d):
            continue
        kb1 = b_cnt[d].max()
        ka1 = np.maximum(a_cnt[d], deg[d] - kb1).max()
        ka2 = a_cnt[d].max()
        kb2 = np.maximum(b_cnt[d], deg[d] - ka2).max()
        if ka1 + kb1 <= ka2 + kb2:
            ka, kb = ka1, kb1
        else:
            ka, kb = ka2, kb2
        KL[t], KH[t] = ka, kb
        nA[d] = np.maximum(a_cnt[d], deg[d] - kb)
    return row, col, deg, nA, starts, dest, KL, KH


def _build_core_arrays(x, c, col, deg, nA, starts, dest, KL, KH):
    """Per-core input arrays (vectorized): packed int16 gather indices,
    softmax mask, destination features."""
    KT = KL + KH
    WLO = int((KL * 8).sum())
    WHI = int((KH * 8).sum())
    WK = int(KT.sum())
    olo8 = np.zeros(TPC, dtype=np.int64)
    ohi8 = np.zeros(TPC, dtype=np.int64)
    ok = np.zeros(TPC, dtype=np.int64)
    np.cumsum(KL[:-1] * 8, out=olo8[1:])
    np.cumsum(KH[:-1] * 8, out=ohi8[1:])
    np.cumsum(KT[:-1], out=ok[1:])

    idxlo = np.zeros((16, max(WLO, 1)), dtype=np.int16)
    idxhi = np.zeros((16, max(WHI, 1)), dtype=np.int16)
    mask = np.full((P, max(WK, 1)), NEG, dtype=np.float32)
    xd = np.zeros((DPC, D), dtype=np.float32)

    slots = np.arange(DPC)
    d_all = dest[c]
    valid = d_all >= 0
    tt, pp = slots // P, slots % P

    xd[valid] = x[d_all[valid]]
    xd[~valid] = x[0]  # finite data for pad dests (output dropped on unshard)

    dv = d_all[valid]
    tv, pv = tt[valid], pp[valid]
    na = nA[dv]  # A-count per dest
    nb = deg[dv] - na

    # --- gather A entries: per dest, cols[starts[d] : starts[d]+na]
    repA = np.repeat(np.arange(len(dv)), na)
    jA = np.arange(repA.size) - np.repeat(np.cumsum(na) - na, na)
    eA = np.repeat(starts[dv], na) + jA
    iA = jA * P + pv[repA]
    cA = olo8[tv[repA]] + iA // 16
    idxlo[iA % 16, cA] = col[eA].astype(np.int16)
    mask[pv[repA], ok[tv[repA]] + jA] = 0.0

    # --- gather B entries: per dest, cols[starts[d]+na : starts[d]+deg]
    repB = np.repeat(np.arange(len(dv)), nb)
    jB = np.arange(repB.size) - np.repeat(np.cumsum(nb) - nb, nb)
    eB = np.repeat(starts[dv] + na, nb) + jB
    iB = jB * P + pv[repB]
    cB = ohi8[tv[repB]] + iB // 16
    idxhi[iB % 16, cB] = (col[eB] - BASE_B).astype(np.int16)
    mask[pv[repB], ok[tv[repB]] + KL[tv[repB]] + jB] = 0.0

    return np.tile(idxlo, (8, 1)), np.tile(idxhi, (8, 1)), mask, xd


# ------------------------------------------------------------- device side


def _build_graph(KL, KH, WLO, WHI, WK):
    import concourse.bass as bass
    import concourse.mybir as mybir
    import concourse.tile as tile
    from concourse import bacc
    from concourse.masks import make_identity

    f32 = mybir.dt.float32
    bf16 = mybir.dt.bfloat16
    i16 = mybir.dt.int16
    AF = mybir.ActivationFunctionType
    OP = mybir.AluOpType

    CHROWS = ACH * P  # table rows per staging chunk
    NCH = (N_NODES + CHROWS - 1) // CHROWS

    nc = bacc.Bacc(num_swdge_queues=2)
    tabx_ext = nc.declare_dram_parameter("tabx", [N_NODES, 256], bf16, isOutput=False)
    xd_ext = nc.declare_dram_parameter("xd", [DPC, D], f32, isOutput=False)
    ilo_ext = nc.declare_dram_parameter("idxlo", [P, max(WLO, 8)], i16, isOutput=False)
    ihi_ext = nc.declare_dram_parameter("idxhi", [P, max(WHI, 8)], i16, isOutput=False)
    mask_ext = nc.declare_dram_parameter("mask", [P, max(WK, 8)], f32, isOutput=False)
    beta_ext = nc.declare_dram_parameter("beta2", [P, 2], f32, isOutput=False)
    out_ext = nc.declare_dram_parameter("out", [TPC, P, P], f32, isOutput=True)

    tab = nc.dram_tensor("tab", [N_NODES, 256], bf16)

    with tile.TileContext(nc) as tc:
        with (
            tc.tile_pool(name="persist", bufs=1) as pp,
            tc.tile_pool(name="chunk", bufs=2) as pch,
            tc.tile_pool(name="xtiles", bufs=3) as pxt,
            tc.tile_pool(name="scr", bufs=2) as psc,
            tc.tile_pool(name="gather", bufs=4) as pg,
            tc.tile_pool(name="small", bufs=3) as psm,
            tc.tile_pool(name="outp", bufs=2) as po,
            tc.tile_pool(name="psum", bufs=4, space="PSUM") as pps,
        ):
            ident = pp.tile([P, P], bf16)
            make_identity(nc, ident[:])
            betat = pp.tile([P, 2], f32)
            nc.sync.dma_start(out=betat[:], in_=beta_ext[:])
            ilo_all = pp.tile([P, max(WLO, 8)], i16)
            nc.sync.dma_start(out=ilo_all[:], in_=ilo_ext[:])
            ihi_all = pp.tile([P, max(WHI, 8)], i16)
            nc.sync.dma_start(out=ihi_all[:], in_=ihi_ext[:])
            mask_all = pp.tile([P, max(WK, 8)], f32)
            nc.sync.dma_start(out=mask_all[:], in_=mask_ext[:])

            # ------- phase A: node table + invnorms (Pool-free) -----------
            # stage tabx in j-major chunks [128, ACH, 256]; row r of chunk ch
            # sits at (p = r%128, j = (r//128)%ACH)
            invbuf = pp.tile([P, NCH * ACH], f32)
            nc.vector.memset(invbuf[:], 1.0)
            # bulk copy tabx -> tab (contiguous DRAM->DRAM)
            nc.scalar.dma_start(out=tab[:], in_=tabx_ext[:])
            for ch in range(NCH):
                r0 = ch * CHROWS
                rows = min(CHROWS, N_NODES - r0)
                nj = (rows + P - 1) // P
                src = tabx_ext[r0 : r0 + rows].rearrange(
                    "(j p) e -> p j e", p=P
                ) if rows % P == 0 else None
                ck = pch.tile([P, ACH, 256], bf16, tag="ck")
                if src is not None:
                    nc.sync.dma_start(out=ck[:, 0:nj, :], in_=src)
                else:
                    full = (rows // P) * P
                    if full:
                        nc.sync.dma_start(
                            out=ck[:, 0 : full // P, :],
                            in_=tabx_ext[r0 : r0 + full].rearrange(
                                "(j p) e -> p j e", p=P
                            ),
                        )
                    rem = rows - full
                    nc.sync.dma_start(
                        out=ck[:rem, full // P : full // P + 1, :],
                        in_=tabx_ext[r0 + full : r0 + rows].rearrange(
                            "(j p) e -> p j e", p=rem
                        ),
                    )
                # per-node sum of squares -> invbuf columns (tile-level)
                sq = psc.tile([P, ACH, D], bf16, tag="sqscr")
                nc.vector.tensor_tensor(
                    out=sq[:, 0:nj, :],
                    in0=ck[:, 0:nj, 0:D],
                    in1=ck[:, 0:nj, 0:D],
                    op=OP.mult,
                )
                nc.vector.tensor_reduce(
                    out=invbuf[:, ch * ACH : ch * ACH + nj],
                    in_=sq[:, 0:nj, :],
                    axis=mybir.AxisListType.X,
                    op=OP.add,
                )
                lg = psm.tile([P, ACH], f32, tag="lg")
                nc.scalar.activation(
                    lg[:, 0:nj], invbuf[:, ch * ACH : ch * ACH + nj], AF.Ln
                )
                inv2 = psm.tile([P, ACH], f32, tag="inv2")
                nc.scalar.activation(
                    inv2[:, 0:nj], lg[:, 0:nj], AF.Exp, scale=-0.5
                )
                # scatter this chunk's invnorms into tab f32 lane 64
                tab_f32 = tab[:].bitcast(f32)
                full = (rows // P) * P
                if full:
                    nc.sync.dma_start(
                        out=tab_f32[r0 : r0 + full, 64:65].rearrange(
                            "(j p) o -> p j o", p=P
                        ),
                        in_=inv2[:, 0 : full // P].unsqueeze(-1),
                    )
                if rows > full:
                    nc.sync.dma_start(
                        out=tab_f32[r0 + full : r0 + rows, 64:65].rearrange(
                            "(j p) o -> p j o", p=rows - full
                        ),
                        in_=inv2[
                            : rows - full, full // P : full // P + 1
                        ].unsqueeze(-1),
                    )


            # ------- phase A2: destination features -----------------------
            xdb = []
            ssd = pp.tile([P, TPC], f32)
            nc.vector.memset(ssd[:], 1.0)
            for t in range(TPC):
                xdt = pxt.tile([P, D], f32, tag="xdt")
                nc.sync.dma_start(out=xdt[:], in_=xd_ext[t * P : (t + 1) * P])
                b = pp.tile([P, D], bf16, tag=f"xdb{t}")
                nc.vector.tensor_copy(b[:], xdt[:])
                xdb.append(b)
                scr = psc.tile([P, D], f32, tag="sqscr")
                nc.scalar.activation(
                    scr[:], xdt[:], AF.Square, accum_out=ssd[:, t : t + 1]
                )
            lgd = pp.tile([P, TPC], f32)
            nc.scalar.activation(lgd[:], ssd[:], AF.Ln)
            invd = pp.tile([P, TPC], f32)
            nc.scalar.activation(invd[:], lgd[:], AF.Exp, scale=-0.5)

            # ------- phase B: per destination tile -------------------------
            olo = ohi = ok = 0
            for t in range(TPC):
                kl, kh = int(KL[t]), int(KH[t])
                kt = kl + kh
                ks = kt + 1  # + self column
                if kt:
                    G = pg.tile([P, kt, 256], bf16, tag="G")
                    if kl:
                        nc.gpsimd.dma_gather(
                            G[:, 0:kl, :], tab[:], ilo_all[:, olo : olo + kl * 8],
                            P * kl, P * kl, 256, single_packet=False,
                            queue_num=0,
                        )
                    if kh:
                        nc.gpsimd.dma_gather(
                            G[:, kl:kt, :], tab[BASE_B:, :],
                            ihi_all[:, ohi : ohi + kh * 8],
                            P * kh, P * kh, 256, single_packet=False,
                            queue_num=1,
                        )
                    mk = mask_all[:, ok : ok + kt]

                cosm = psm.tile([P, ks], f32, tag="cosm")
                nc.vector.memset(cosm[:, kt : kt + 1], 1.0)  # self cos
                if kt:
                    # tile-level dot products
                    prod = psc.tile([P, kt, D], bf16, tag="prod")
                    nc.vector.tensor_tensor(
                        out=prod[:],
                        in0=G[:, :, 0:D],
                        in1=xdb[t][:, None, :].broadcast_to([P, kt, D]),
                        op=OP.mult,
                    )
                    dotr = psm.tile([P, kt], f32, tag="dotr")
                    nc.vector.tensor_reduce(
                        out=dotr[:], in_=prod[:], axis=mybir.AxisListType.X,
                        op=OP.add,
                    )
                    # cos = dotr * invd (per-partition) * invc (gathered)
                    invc = G[:, :, 128:130].bitcast(f32)  # [P, kt, 1]
                    nc.vector.scalar_tensor_tensor(
                        out=cosm[:, 0:kt].unsqueeze(-1),
                        in0=dotr[:].unsqueeze(-1),
                        scalar=invd[:, t : t + 1],
                        in1=invc,
                        op0=OP.mult,
                        op1=OP.mult,
                    )
                    nc.vector.tensor_tensor(
                        out=cosm[:, 0:kt], in0=cosm[:, 0:kt], in1=mk, op=OP.add
                    )
                # softmax over ks columns
                mx = psm.tile([P, 1], f32, tag="mx")
                nc.vector.tensor_reduce(
                    out=mx[:], in_=cosm[:], axis=mybir.AxisListType.X, op=OP.max
                )
                nbm = psm.tile([P, 1], f32, tag="nbm")
                nc.vector.tensor_scalar(
                    out=nbm[:], in0=mx[:], scalar1=betat[:, 1:2], scalar2=None,
                    op0=OP.mult,
                )
                w = psm.tile([P, ks], f32, tag="w")
                nc.scalar.activation(
                    w[:], cosm[:], AF.Exp, bias=nbm[:], scale=betat[:, 0:1]
                )
                dn = psm.tile([P, 1], f32, tag="dn")
                nc.vector.tensor_reduce(
                    out=dn[:], in_=w[:], axis=mybir.AxisListType.X, op=OP.add
                )
                ivn = psm.tile([P, 1], f32, tag="ivn")
                nc.vector.reciprocal(ivn[:], dn[:])
                wn = psm.tile([P, ks], f32, tag="wn")
                nc.vector.tensor_scalar(
                    out=wn[:], in0=w[:], scalar1=ivn[:], scalar2=None, op0=OP.mult
                )
                # diag blocks: diag[:, j*128+q] = (q==p) * wn[p, j]
                diag = psc.tile([P, ks, P], bf16, tag="diag")
                nc.vector.tensor_tensor(
                    out=diag[:],
                    in0=ident[:, None, :].broadcast_to([P, ks, P]),
                    in1=wn[:].unsqueeze(-1).broadcast_to([P, ks, P]),
                    op=OP.mult,
                )
                # aggregation: psum[f, d] += sum_e G_j[e, f] * diag_j[e, d]
                ps = pps.tile([P, P], f32)
                for j in range(kt):
                    nc.tensor.matmul(
                        out=ps[:], lhsT=G[:, j, 0:D], rhs=diag[:, j, :],
                        start=(j == 0), stop=False,
                    )
                nc.tensor.matmul(
                    out=ps[:], lhsT=xdb[t][:], rhs=diag[:, kt, :],
                    start=(kt == 0), stop=True,
                )
                ob = po.tile([P, P], f32, tag="ob")
                nc.scalar.copy(ob[:], ps[:])
                nc.sync.dma_start(out=out_ext[t], in_=ob[:])

                olo += kl * 8
                ohi += kh * 8
                ok += kt
    nc.finalize()
    return nc


# ----------------------------------------------------------------- entry


def kernel(x, beta, edge_index):
    import ml_dtypes

    from concourse.bass_utils import run_bass_kernel_spmd

    x = np.asarray(x, dtype=np.float32)
    beta = np.asarray(beta, dtype=np.float32)

    row, col, deg, nA, starts, dest, KL, KH = _preprocess(edge_index)
    KT = KL + KH
    WLO = int((KL * 8).sum())
    WHI = int((KH * 8).sum())
    WK = int(KT.sum())

    beta2 = np.zeros((P, 2), dtype=np.float32)
    beta2[:, 0] = beta[0]
    beta2[:, 1] = -beta[0]

    tabx = np.zeros((N_NODES, 256), dtype=ml_dtypes.bfloat16)
    tabx[:, 0:D] = x.astype(ml_dtypes.bfloat16)

    in_maps = []
    for c in range(NC):
        idxlo, idxhi, mask, xd = _build_core_arrays(
            x, c, col, deg, nA, starts, dest, KL, KH
        )
        if WLO == 0:
            idxlo = np.zeros((P, 8), dtype=np.int16)
        if WHI == 0:
            idxhi = np.zeros((P, 8), dtype=np.int16)
        if WK == 0:
            mask = np.zeros((P, 8), dtype=np.float32)
        in_maps.append(
            {
                "tabx": tabx,
                "xd": xd,
                "idxlo": idxlo,
                "idxhi": idxhi,
                "mask": mask,
                "beta2": beta2,
            }
        )

    nc = _build_graph(KL, KH, WLO, WHI, WK)
    import os

    trace = bool(int(os.environ.get("KERNEL_TRACE", "0")))
    res = run_bass_kernel_spmd(
        nc, in_maps, core_ids=list(range(NC)), trace=trace
    )
    global _last_results
    _last_results = res

    out = np.zeros((N_NODES, D), dtype=np.float32)
    for c in range(NC):
        o = res.results[c]["out"].reshape(TPC, P, P)  # [t, feat, dest]
        for t in range(TPC):
            d = dest[c, t * P : (t + 1) * P]
            live = d >= 0
            out[d[live]] = o[t][:, live].T
    return out


if __name__ == "__main__":
    sys.path.insert(0, "/root/problem")
    import reference

    inputs = {k: np.asarray(v) for k, v in reference.setup_inputs().items()}
    expected = np.asarray(reference.reference(**inputs))
    actual = kernel(**inputs)
    rel = np.linalg.norm(actual - expected) / np.linalg.norm(expected)
    print("rel:", rel)

